# revision 2
# baseline (speedup 1.0000x reference)
"""TRN2 Bass kernel for nn_Decoder_83279415870148 — v6.

6-layer causal decoder (B=8, S=1024, D=512, H=8, DFF=2048), data-parallel:
one batch element per core, weights replicated, no collectives.

v4: fp8-DoubleRow scores (q/k shuffled to pair layout via SBUF->SBUF DMA),
causal tri-mask as rank-128 fp8 DR matmul, LN stats via f32r-sum +
fp8-square-DR-sumsq matmuls (no bf16 copy pass), all bias matmuls folded
into LN write columns host-side, MLP bf16. DMA restructured: one merged DMA
per weight tensor per layer (6/layer vs 31), emission-positioned so the
in-order SP queue never head-of-line blocks; q+k shuffle combined (4/layer
per head-pair). PSUM: 1-bank tiles, deep pipelining (bufs=6).
Layout: activations transposed [feature -> partitions, tokens -> free].
"""
import numpy as np
import ml_dtypes
from contextlib import ExitStack

import concourse.bass as bass
import concourse.tile as tile
from concourse import bacc, mybir
from concourse.bass_utils import run_bass_kernel_spmd

P = 128
B, S, D, H, L = 8, 1024, 512, 8, 6
DK = D // H
DFF = 4 * D
DC = D // P          # 4
FC = DFF // P        # 16
NT = S // P          # 8
TH = 512
EPS = 1e-5
BF = mybir.dt.bfloat16
F8 = mybir.dt.float8e4
F32 = mybir.dt.float32
F32R = mybir.dt.float32r
AF = mybir.ActivationFunctionType
DR = mybir.MatmulPerfMode.DoubleRow
OP = mybir.AluOpType

S_W = 512.0
S_X = 16.0
S_V = 32.0
S_ATT = 32.0
S_O = 32.0
S_Q8 = 16.0
S_K8 = 16.0
C_QK = S_Q8 / (S_X * S_W)
C_K = S_K8 / (S_X * S_W)
SV_DS = S_V / (S_X * S_W)
DS_O = 1.0 / (S_O * S_W)
SC_EXP = 1.0 / (8.0 * S_Q8 * S_K8)
LN_SATT = float(np.log(S_ATT))
VW = H * 66
SPANS = [S, S - 256, S - 512, S - 768]
N_RELU_DVE = 0       # fc chunks 0..7 evict on DVE (shifted relu), rest ACT
NBIAS = DC + 2 * FC + 10 * DC  # merged bias tile columns


def build(nlayers=L):
    nc = bacc.Bacc("TRN2", target_bir_lowering=False, debug=False, num_devices=8)
    dt = nc.dram_tensor
    x8_d = dt("x8", [2, P, 2 * S], F8, kind="ExternalInput").ap()
    xT32_d = dt("xT32", [D, S], F32, kind="ExternalInput").ap()
    wqk8_d = dt("wqk8", [L, P, 2 * 2 * 2 * D], F8, kind="ExternalInput").ap()
    wvo8_d = dt("wvo8", [L, P, 2 * 2 * 2 * D], F8, kind="ExternalInput").ap()
    w1_d = dt("w1", [L, P, DC * DFF], BF, kind="ExternalInput").ap()
    w2_d = dt("w2", [L, P, FC * D], BF, kind="ExternalInput").ap()
    bias_d = dt("bias", [L, P, NBIAS], F32, kind="ExternalInput").ap()
    tri_d = dt("tri", [64, 2 * 2 * P], F8, kind="ExternalInput").ap()
    out_d = dt("outT", [D, S], F32, kind="ExternalOutput").ap()

    with tile.TileContext(nc) as tc, ExitStack() as ctx:
        cp = ctx.enter_context(tc.tile_pool(name="cp", bufs=1))
        wp = ctx.enter_context(tc.tile_pool(name="wp", bufs=1))
        ap = ctx.enter_context(tc.tile_pool(name="ap", bufs=1))
        psA = ctx.enter_context(tc.tile_pool(name="psA", bufs=2, space="PSUM"))
        psB = ctx.enter_context(tc.tile_pool(name="psB", bufs=2, space="PSUM"))

        # ---------------- consts ----------------
        it8 = cp.tile([64, 4 * P], F8, name="it8")
        nc.sync.dma_start(it8[:], tri_d)
        i83 = it8[:].rearrange("p (g m) -> p g m", g=4)[:, 0:2]
        t83 = it8[:].rearrange("p (g m) -> p g m", g=4)[:, 2:4]
        ones8 = cp.tile([P, 2 * P], F8, name="ones8")
        nc.vector.memset(ones8[:], 1.0)
        ones32 = cp.tile([P, P], F32, name="ones32")
        nc.vector.memset(ones32[:], 1.0)
        ones_r = cp.tile([P, P], F32R, name="ones_r")
        nc.vector.tensor_copy(ones_r[:], ones32[:])
        eps_t = cp.tile([P, 1], F32, name="eps_t")
        nc.vector.memset(eps_t[:], EPS)
        lnsc_t = cp.tile([P, 1], F32, name="lnsc_t")
        nc.vector.memset(lnsc_t[:], LN_SATT)
        o83 = ones8[:].rearrange("p (two m) -> p two m", two=2)

        # ---------------- persistent activations ----------------
        x8 = [cp.tile([P, 2 * S], F8, name=f"x8_{u}") for u in range(2)]
        xa = [cp.tile([P, S], F32, name=f"xa{c}") for c in range(DC)]
        x1b = [cp.tile([P, S], BF, name=f"x1b{c}") for c in range(DC)]
        oT8 = [cp.tile([P, 2 * S], F8, name=f"oT8_{u}") for u in range(2)]
        vp = [cp.tile([P, 2 * VW], F8, name=f"vp{u}") for u in range(4)]
        ptm = {}
        for bb in range(2):
            for u in range(4):
                t = cp.tile([P, 4 * SPANS[u]], F8, name=f"ptm{bb}{u}")
                v4_ = t[:].rearrange("p (sub two m) -> p sub two m",
                                     sub=2, two=2)
                nc.vector.memset(v4_[:, :, 1, 0:P], 0.0)
                ptm[(bb, u)] = t
        for u in range(4):
            nc.vector.memset(vp[u][:], 0.0)
            v3 = vp[u][:].rearrange("p (two h c) -> p two h c", two=2, h=H)
            nc.vector.memset(v3[:, :, :, 64:65], 1.0)

        for u in range(2):
            nc.sync.dma_start(x8[u][:], x8_d[u])
        for c in range(DC):
            nc.sync.dma_start(xa[c][:], xT32_d[c * P:(c + 1) * P])

        # -------- weight loaders (one merged DMA each) --------
        def load_wqk(i):
            t = wp.tile([P, 8 * D], F8, name=f"wqk{i}", tag="wqk")
            nc.sync.dma_start(t[:], wqk8_d[i])
            return t

        def load_wvo(i):
            t = wp.tile([P, 8 * D], F8, name=f"wvo{i}", tag="wvo")
            nc.sync.dma_start(t[:], wvo8_d[i])
            return t

        def load_w1(i):
            t = wp.tile([P, DC * DFF], BF, name=f"w1_{i}", tag="w1")
            nc.sync.dma_start(t[:], w1_d[i])
            return t

        def load_w2(i):
            t = wp.tile([P, FC * D], BF, name=f"w2_{i}", tag="w2")
            nc.sync.dma_start(t[:], w2_d[i])
            return t

        def load_bias(i):
            t = wp.tile([P, NBIAS], F32, name=f"bias{i}", tag="bias", bufs=2)
            nc.sync.dma_start(t[:], bias_d[i])
            return t

        cur = {"wqk": load_wqk(0), "wvo": load_wvo(0), "w1": load_w1(0),
               "w2": load_w2(0), "bias": load_bias(0)}

        for i in range(nlayers):
            last = (i == nlayers - 1)
            wqk, wvo = cur["wqk"], cur["wvo"]
            w1t, w2t, bias_t = cur["w1"], cur["w2"], cur["bias"]
            nxt = {}
            if not last:
                nxt["bias"] = load_bias(i + 1)

            # views: wqk [P, (u 2, qk 2, two 2, D)]
            wqk4 = wqk[:].rearrange("p (u qk two m) -> p u qk two m",
                                    u=2, qk=2, two=2)
            wvo4 = wvo[:].rearrange("p (u vo two m) -> p u vo two m",
                                    u=2, vo=2, two=2)
            x83 = [x8[u][:].rearrange("p (two m) -> p two m", two=2)
                   for u in range(2)]
            bq_c = lambda mc: bias_t[:, mc:mc + 1]
            b1a_c = lambda fc: bias_t[:, DC + fc:DC + fc + 1]
            b1d_c = lambda fc: bias_t[:, DC + FC + fc:DC + FC + fc + 1]
            ln_c = lambda g, mc: bias_t[:, DC + 2 * FC + g * DC + mc:
                                        DC + 2 * FC + g * DC + mc + 1]

            qkp = {}

            def emit_qk(ti):
                qk8l = ap.tile([P, 2 * S], F8, name=f"qk8l{ti}", tag="qk8l",
                               bufs=2)
                for qk, (scale, bias) in enumerate(
                        ((C_QK, bq_c(ti)), (C_K, None))):
                    for hf in range(2):
                        sl = slice(hf * TH, (hf + 1) * TH)
                        pt = psA.tile([P, TH], F32, name="pQK", tag="mm")
                        for u in range(2):
                            nc.tensor.matmul(pt[:],
                                             wqk4[:, u, qk, :,
                                                  ti * P:(ti + 1) * P],
                                             x83[u][:, :, sl], start=(u == 0),
                                             stop=(u == 1), perf_mode=DR)
                        o = qk8l[:, qk * S + hf * TH:qk * S + (hf + 1) * TH]
                        if bias is not None:
                            nc.vector.tensor_scalar(o, pt[:], scale, bias,
                                                    OP.mult, OP.add)
                        else:
                            nc.vector.tensor_scalar_mul(o, pt[:], scale)
                qkp8 = ap.tile([32, 8 * S], F8, name=f"qkp8{ti}", tag="qkp8",
                               bufs=2)
                for g in range(4):
                    nc.sync.dma_start(qkp8[:, g * 2 * S:(g + 1) * 2 * S],
                                      qk8l[g * 32:(g + 1) * 32, :])
                qkp[ti] = qkp8

            def emit_v():
                for w in range(4):
                    for tpar in range(2):
                        t = 2 * w + tpar
                        pv = psA.tile([P, D], F32, name="pV", tag="mm")
                        for u in range(2):
                            nc.tensor.matmul(pv[:],
                                             x83[u][:, :, t * P:(t + 1) * P],
                                             wvo4[:, u, 0],
                                             start=(u == 0), stop=(u == 1),
                                             perf_mode=DR)
                        v3 = vp[w][:].rearrange("p (two h c) -> p two h c",
                                                two=2, h=H)
                        nc.vector.tensor_scalar_mul(
                            v3[:, tpar, :, 0:64],
                            pv[:].rearrange("p (h c) -> p h c", h=H), SV_DS)

            def emit_scores(hp):
                bb = hp % 2
                qk4 = qkp[hp][:].rearrange("p (g qk s) -> p g qk s", g=4, qk=2)
                for j in range(NT):
                    u, half = j // 2, j % 2
                    q0 = j * P
                    rem = S - q0
                    s1 = min(TH, rem)
                    spans = [(q0, s1)]
                    if rem > s1:
                        spans.append((q0 + s1, rem - s1))
                    p4 = ptm[(bb, u)][:].rearrange(
                        "p (sub two m) -> p sub two m", sub=2, two=2)
                    for (qs, sl_len) in spans:
                        first = (qs == q0)
                        ps = psA.tile([P, 2 * TH], F32, name="pS", tag="mm")
                        for sub in range(2):
                            nc.tensor.matmul(
                                ps[:, sub * TH:sub * TH + sl_len],
                                qk4[:, 2 * sub:2 * sub + 2, 1, q0:q0 + P],
                                qk4[:, 2 * sub:2 * sub + 2, 0, qs:qs + sl_len],
                                start=True, stop=not first, perf_mode=DR)
                            if first:
                                nc.tensor.matmul(
                                    ps[:, sub * TH:sub * TH + P], i83, t83,
                                    start=False, stop=True, perf_mode=DR,
                                    skip_group_check=True)
                        co = qs - 256 * u
                        nc.scalar.activation(
                            p4[:, :, half, co:co + sl_len],
                            ps[:].rearrange("p (sub m) -> p sub m",
                                            sub=2)[:, :, 0:sl_len],
                            AF.Exp, scale=SC_EXP, bias=lnsc_t[:])

            def emit_av(hp):
                bb = hp % 2
                for sub in range(2):
                    h = 2 * hp + sub
                    u2, hh, ro = h // 4, (h // 2) % 2, 64 * (h % 2)
                    for hf in range(2):
                        nu = 2 if hf == 0 else 4
                        po = psB.tile([66, TH], F32, name="pO", tag="po")
                        pd = psB.tile([64, TH], F32, name="pD", tag="pd")
                        for u in range(nu):
                            rs = max(hf * TH, u * 256)
                            pt3 = ptm[(bb, u)][:].rearrange(
                                "p (sub two m) -> p sub two m",
                                sub=2, two=2)[:, sub]
                            v3 = vp[u][:].rearrange(
                                "p (two m) -> p two m", two=2)
                            rhs = pt3[:, :, rs - 256 * u:
                                      (hf + 1) * TH - 256 * u]
                            nc.tensor.matmul(
                                po[:, rs - hf * TH:TH],
                                v3[:, :, h * 66:h * 66 + 66],
                                rhs, start=(u == 0), stop=(u == nu - 1),
                                perf_mode=DR, skip_group_check=(u > 0))
                            nc.tensor.matmul(
                                pd[:, rs - hf * TH:TH],
                                o83[:, :, 0:64],
                                rhs, start=(u == 0), stop=(u == nu - 1),
                                perf_mode=DR, skip_group_check=(u > 0))
                        rbs = ap.tile([64, TH], F32, name="rbs", tag="rbs",
                                      bufs=2)
                        nc.vector.reciprocal(rbs[:], pd[:])
                        nc.vector.scalar_tensor_tensor(
                            oT8[u2][ro:ro + 64,
                                    hh * S + hf * TH:hh * S + (hf + 1) * TH],
                            po[0:64, :], S_O / S_V, rbs[:], OP.mult, OP.mult)

            # ---------------- attention phases ----------------
            emit_qk(0)
            emit_qk(1)
            emit_scores(0)
            emit_qk(2)
            emit_v()
            emit_scores(1)
            emit_qk(3)
            if not last:
                nxt["wqk"] = load_wqk(i + 1)
            emit_av(0)
            emit_scores(2)
            emit_av(1)
            emit_scores(3)
            emit_av(2)
            emit_av(3)

            # ---------------- LN helper (per half) ----------------
            def ln_half(hf, ysrc, dst8, dst8_cols, dstbf, dstbf_cols,
                        dst32, dst32_cols, dst32_pool=True):
                sl = slice(hf * TH, (hf + 1) * TH)
                sq8 = [ap.tile([P, 2 * TH], F8, name=f"sq8_{w}", tag=f"sq8_{w}")
                       for w in range(2)]
                for mc in range(DC):
                    o = sq8[mc // 2][:].rearrange(
                        "p (two m) -> p two m", two=2)[:, mc % 2, :]
                    nc.gpsimd.tensor_tensor(o, ysrc[mc][:], ysrc[mc][:],
                                            OP.mult)
                ps_s = psA.tile([P, TH], F32, name="lnS", tag="mm")
                for mc in range(DC):
                    nc.tensor.matmul(ps_s[:], ones_r[:], ysrc[mc][:],
                                     start=(mc == 0), stop=(mc == DC - 1))
                ps_q = psA.tile([P, TH], F32, name="lnQ", tag="mm")
                for w in range(2):
                    nc.tensor.matmul(ps_q[:], o83,
                                     sq8[w][:].rearrange(
                                         "p (two m) -> p two m", two=2),
                                     start=(w == 0), stop=(w == 1),
                                     perf_mode=DR)
                # stats tail: Square reads the raw sum psum (no stA dep)
                stT = ap.tile([P, TH], F32, name="stT", tag="stT")
                nc.scalar.activation(stT[:], ps_s[:], AF.Square, scale=1.0 / D)
                stA = ap.tile([P, TH], F32, name="stA", tag="stA", bufs=2)
                nc.scalar.activation(stA[:], ps_s[:], AF.Identity,
                                     scale=1.0 / D)
                stD = ap.tile([P, TH], F32, name="stD", tag="stD")
                nc.vector.scalar_tensor_tensor(stD[:], ps_q[:], 1.0 / D,
                                               stT[:], OP.mult, OP.subtract)
                stS = ap.tile([P, TH], F32, name="stS", tag="stS", bufs=2)
                nc.scalar.activation(stS[:], stD[:], AF.Sqrt, bias=eps_t[:])
                rstd = ap.tile([P, TH], F32, name="rstd", tag="rstd", bufs=2)
                nc.vector.reciprocal(rstd[:], stS[:])
                lts = []
                for mc in range(DC):
                    lt = ap.tile([P, TH], F32, name="lt", tag=f"lt{mc}")
                    nc.vector.tensor_tensor(lt[:], ysrc[mc][:], stA[:],
                                            OP.subtract)
                    lts.append(lt)
                for mc in range(DC):
                    lu = ap.tile([P, TH], F32, name="lu", tag=f"lu{mc}")
                    nc.vector.tensor_tensor(lu[:], lts[mc][:], rstd[:],
                                            OP.mult)
                    if dst8 is not None:
                        g8, b8 = dst8_cols
                        o8 = dst8[mc // 2][:].rearrange(
                            "p (two m) -> p two m", two=2)[:, mc % 2, sl]
                        nc.gpsimd.tensor_scalar(
                            o8, lu[:], ln_c(g8, mc), ln_c(b8, mc),
                            OP.mult, OP.add)
                    if dstbf is not None:
                        gb, bb_ = dstbf_cols
                        nc.vector.tensor_scalar(
                            dstbf[mc][:, sl], lu[:],
                            ln_c(gb, mc), ln_c(bb_, mc), OP.mult, OP.add)
                    if dst32 is not None:
                        g32, b32 = dst32_cols
                        if dst32_pool:
                            nc.gpsimd.tensor_scalar(
                                dst32[mc][:, sl], lu[:],
                                ln_c(g32, mc), ln_c(b32, mc), OP.mult, OP.add)
                        else:
                            nc.vector.tensor_scalar(
                                dst32[mc][:, sl], lu[:],
                                ln_c(g32, mc), ln_c(b32, mc), OP.mult, OP.add)

            # ------- phase D (both halves), then LNs, then MLP -------
            yh = {}
            for hf in range(2):
                sl = slice(hf * TH, (hf + 1) * TH)
                y32h = []
                for mc in range(DC):
                    pt = psA.tile([P, TH], F32, name="pP", tag="mm")
                    for u in range(2):
                        rhs = oT8[u][:].rearrange("p (two m) -> p two m", two=2)
                        nc.tensor.matmul(pt[:],
                                         wvo4[:, u, 1, :, mc * P:(mc + 1) * P],
                                         rhs[:, :, sl], start=(u == 0),
                                         stop=(u == 1), perf_mode=DR)
                    yt = ap.tile([P, TH], F32R, name=f"y_{hf}_{mc}",
                                 tag=f"y_{hf}_{mc}")
                    nc.vector.scalar_tensor_tensor(yt[:], pt[:], DS_O,
                                                   xa[mc][:, sl],
                                                   OP.mult, OP.add)
                    y32h.append(yt)
                yh[hf] = y32h
            if not last:
                nxt["wvo"] = load_wvo(i + 1)
                nxt["w2"] = load_w2(i + 1)
            ln_half(0, yh[0], None, None, x1b, (0, 1), None, None)
            ln_half(1, yh[1], None, None, x1b, (0, 1), None, None)

            hts_h = {}
            for hf in range(2):
                sl = slice(hf * TH, (hf + 1) * TH)
                hts = []
                for fc in range(FC):
                    ph = psA.tile([P, TH], F32, name="pH", tag="mm")
                    for kc in range(DC):
                        nc.tensor.matmul(ph[:],
                                         w1t[:, kc * DFF + fc * P:
                                             kc * DFF + (fc + 1) * P],
                                         x1b[kc][:, sl],
                                         start=(kc == 0), stop=(kc == DC - 1))
                    ht = ap.tile([P, TH], BF, name=f"ht{fc}", tag=f"h{fc}",
                                 bufs=2)
                    if fc < N_RELU_DVE:
                        nc.vector.tensor_scalar(
                            ht[:], ph[:], 1.0, b1d_c(fc), OP.mult, OP.max)
                    else:
                        nc.scalar.activation(ht[:], ph[:], AF.Relu,
                                             bias=b1a_c(fc))
                    hts.append(ht)
                hts_h[hf] = hts
            if not last:
                nxt["w1"] = load_w1(i + 1)
            y2h_h = {}
            for hf in range(2):
                sl = slice(hf * TH, (hf + 1) * TH)
                y2h = []
                for mc in range(DC):
                    pt = psA.tile([P, TH], F32, name="pY", tag="mm")
                    for fc in range(FC):
                        nc.tensor.matmul(pt[:],
                                         w2t[:, fc * D + mc * P:
                                             fc * D + (mc + 1) * P],
                                         hts_h[hf][fc][:], start=(fc == 0),
                                         stop=(fc == FC - 1))
                    yt = ap.tile([P, TH], F32R, name=f"y2_{hf}_{mc}",
                                 tag=f"y_{hf}_{mc}")
                    nc.vector.scalar_tensor_tensor(yt[:], pt[:],
                                                   ln_c(2, mc),
                                                   x1b[mc][:, sl],
                                                   OP.add, OP.add)
                    y2h.append(yt)
                y2h_h[hf] = y2h
            for hf in range(2):
                ln_half(hf, y2h_h[hf], None if last else x8, (4, 5),
                        None, None, xa, (8, 9) if last else (6, 7))

            cur = nxt

        for c in range(DC):
            nc.sync.dma_start(out_d[c * P:(c + 1) * P], xa[c][:])

    nc.compile()
    return nc


_CACHE = {}


def _f8c(a, s):
    a = np.ascontiguousarray(np.asarray(a), dtype=np.float32) * s
    return np.clip(a, -240.0, 240.0).astype(ml_dtypes.float8_e4m3)


def _pairs(w, s):
    # w [K, M] fp32 -> [K//256, 128, 2*M] fp8: tile u = K-chunks (2u, 2u+1)
    K, M = w.shape
    r = np.asarray(w, np.float32).reshape(K // P // 2, 2, P, M)
    r = np.transpose(r, (0, 2, 1, 3)).reshape(K // P // 2, P, 2 * M)
    return _f8c(r, s)


def _host_prep(qkv_w, qkv_b, out_w, out_b, ln1_g, ln1_b, mlp_w1, mlp_b1,
               mlp_w2, mlp_b2, ln2_g, ln2_b):
    f32c = lambda a: np.ascontiguousarray(np.asarray(a), dtype=np.float32)
    qkv_w = np.asarray(qkv_w)
    bo_eff = np.asarray(out_b) + np.einsum(
        "ld,ldo->lo", np.asarray(qkv_b)[:, 2 * D:].astype(np.float64),
        np.asarray(out_w).astype(np.float64)).astype(np.float32)
    b1 = np.asarray(mlp_b1, np.float64)
    b1_dve = b1.copy()
    b1_dve[:, N_RELU_DVE * P:] = 0.0
    b2_eff = np.asarray(mlp_b2) + np.einsum(
        "lf,lfd->ld", b1_dve, np.asarray(mlp_w2).astype(np.float64)
    ).astype(np.float32)
    g1 = np.asarray(ln1_g, np.float32); be1 = np.asarray(ln1_b, np.float32)
    g2 = np.asarray(ln2_g, np.float32); be2 = np.asarray(ln2_b, np.float32)
    bo_next = np.zeros_like(be2)
    bo_next[:L - 1] = bo_eff[1:]
    lnp = np.stack([
        g1, be1,                        # x1b (bf16 MLP input)
        b2_eff, np.zeros_like(b2_eff),  # y2-evict bias col (residual=x1b)
        g2 * S_X, be2 * S_X,            # x8' (fp8 QKV input)
        g2, be2 + bo_next,              # xa' (attn residual, bo folded)
        g2, be2,                        # last-layer output
    ], axis=1).astype(np.float32)     # [L, 10, D]
    b1f = np.asarray(mlp_b1, np.float32)
    # merged bias tile [L, P, NBIAS]: bq cols | b1_act FC | b1_dve FC | ln 10*DC
    bias = np.zeros((L, P, NBIAS), np.float32)
    bias[:, :, :DC] = (np.asarray(qkv_b)[:, :D] * S_Q8).reshape(L, DC, P
                                                               ).transpose(0, 2, 1)
    bias[:, :, DC:DC + FC] = b1f.reshape(L, FC, P).transpose(0, 2, 1)
    bias[:, :, DC + FC:DC + 2 * FC] = (-b1f).reshape(L, FC, P
                                                    ).transpose(0, 2, 1)
    bias[:, :, DC + 2 * FC:] = lnp.reshape(L, 10, DC, P).transpose(
        0, 3, 1, 2).reshape(L, P, 10 * DC)
    # wqk merged: [L, P, (u 2, qk 2, two 2, D)]
    wq = np.stack([_pairs(qkv_w[l, :, :D], S_W) for l in range(L)])    # [L,2,P,2D]
    wk = np.stack([_pairs(qkv_w[l, :, D:2 * D], S_W) for l in range(L)])
    wv = np.stack([_pairs(qkv_w[l, :, 2 * D:], S_W) for l in range(L)])
    wo = np.stack([_pairs(np.asarray(out_w)[l], S_W) for l in range(L)])
    wqk8 = np.stack([wq, wk], axis=2)     # [L, 2(u), 2(qk), P, 2D]
    wqk8 = wqk8.transpose(0, 3, 1, 2, 4).reshape(L, P, 8 * D)
    wvo8 = np.stack([wv, wo], axis=2)
    wvo8 = wvo8.transpose(0, 3, 1, 2, 4).reshape(L, P, 8 * D)
    # w1 [L, P, DC*DFF]: chunk kc at cols kc*DFF..(kc+1)*DFF
    w1b = np.asarray(mlp_w1, np.float32).reshape(L, DC, P, DFF).transpose(
        0, 2, 1, 3).reshape(L, P, DC * DFF).astype(ml_dtypes.bfloat16)
    w2b = np.asarray(mlp_w2, np.float32).reshape(L, FC, P, D).transpose(
        0, 2, 1, 3).reshape(L, P, FC * D).astype(ml_dtypes.bfloat16)
    # tri consts packed [64, (g 4, P)]: g 0,1 = ident8 pair, g 2,3 = tri pair
    i8 = np.zeros((64, 2, P), np.float32)
    t8 = np.zeros((64, 2, P), np.float32)
    for p in range(64):
        for b in range(2):
            d = p + 64 * b
            i8[p, b, d] = 240.0
            t8[p, b, :d] = -176.0
    tri = np.concatenate([i8, t8], axis=1).reshape(64, 4 * P)
    common = {
        "wqk8": np.ascontiguousarray(wqk8),
        "wvo8": np.ascontiguousarray(wvo8),
        "w1": np.ascontiguousarray(w1b),
        "w2": np.ascontiguousarray(w2b),
        "bias": bias,
        "tri": tri.astype(ml_dtypes.float8_e4m3),
    }
    return common, bo_eff


def kernel(x, qkv_w, qkv_b, out_w, out_b, ln1_g, ln1_b, mlp_w1, mlp_b1,
           mlp_w2, mlp_b2, ln2_g, ln2_b):
    if "nc" not in _CACHE:
        _CACHE["nc"] = build()
    nc = _CACHE["nc"]
    common, bo_eff = _host_prep(qkv_w, qkv_b, out_w, out_b, ln1_g, ln1_b,
                                mlp_w1, mlp_b1, mlp_w2, mlp_b2, ln2_g, ln2_b)
    x = np.asarray(x)
    in_maps = []
    for c in range(B):
        xt = np.ascontiguousarray(x[c].T).astype(np.float32)
        in_maps.append({**common,
                        "x8": _pairs(xt, S_X),
                        "xT32": xt + bo_eff[0][:, None]})
    res = run_bass_kernel_spmd(nc, in_maps, core_ids=list(range(B)))
    out = np.stack([np.ascontiguousarray(r["outT"].T) for r in res.results])
    return out.astype(np.float32)
